# revision 9
# baseline (speedup 1.0000x reference)
"""Trainium2 Bass kernel for a ViT-style transformer block (pre-norm MHA + MLP).

Sharding: pure data-parallel over batch. 16 images -> 8 cores x 2 images.
No collectives. Each core runs an identical SPMD program on its 2 images.

Numerics: matmuls run in bf16 (stationary bf16 gets FWL fast-weight-load;
fp32r's forced ~209ns self-loading LDWEIGHTS per matmul was the bottleneck),
PSUM accumulation is fp32, the residual stream / LN statistics / softmax
denominators stay fp32 (denominator broadcast + odd-head shift matmuls in
fp32r). All weights are bf16-resident in SBUF (no streaming).

Device-side dataflow (per core, tokens 2x577 padded per-batch to 2x640):
  - x resident fp32 token-major [128, 10, 768].
  - LN1 token-major (DVE stats, ACT apply -> bf16 h), PE-transpose h ->
    h^T bf16 feature-major [128, 6, 1280].
  - QK^T feature-major [128, 12, 2, 578] bf16; V token-major per
    (batch, tile, head) [128, 2, 5, 12, 66] bf16 with a ones column
    (col 64; zeroed on pad tokens) that yields the softmax denominator.
  - Attention per (batch, head): S^T = K^T.T @ Q^T -> psum, exp on ACT ->
    es bf16; [V|1].T @ es accumulated over k-chunks -> psum [66, 578]:
    rows 0:64 = O^T, row 64 = denominator. Denominator row is
    partition-broadcast via a K=1 fp32r ones-matmul (PE is the only
    partition-moving engine), fast-reciprocal on DVE, normalize fused
    into the psum->sbuf copy. Odd heads move to partitions 64:128 via an
    fp32r shift-matmul.
  - proj token-major, residual added by DVE in-place into fp32 x.
  - LN2 + transpose, then MLP: fc1 feature-major per 128-row hidden
    chunk, gelu(+bias) on ACT, fc2 token-major accumulated in psum over
    the 24 hidden chunks, fp32 residual add -> y.
"""

import os
import sys

import numpy as np

_TRN_REPO = "/opt/trn_rl_repo"
if os.path.isdir(_TRN_REPO) and _TRN_REPO not in sys.path:
    try:
        import concourse  # noqa: F401
    except ImportError:
        sys.path.insert(0, _TRN_REPO)

import ml_dtypes  # noqa: E402
import concourse.bass as bass  # noqa: E402
import concourse.mybir as mybir  # noqa: E402
import concourse.tile as tile  # noqa: E402
from concourse import bacc  # noqa: E402
from concourse.masks import make_identity  # noqa: E402

F32 = mybir.dt.float32
F32R = mybir.dt.float32r
BF16 = mybir.dt.bfloat16
AF = mybir.ActivationFunctionType
AX = mybir.AxisListType

DIM = 768
HEADS = 12
HD = 64
HIDDEN = 3072
B = 16
N = 577
CORES = 8
BPC = B // CORES          # batches per core = 2
NB = 640                  # padded tokens per batch (5 * 128)
NT = 5                    # token tiles per batch
TT = BPC * NT             # token tiles per core = 10
TOKP = BPC * NB           # 1280
KC = DIM // 128           # 6 contraction chunks over model dim
MC_QK = 12                # 128-row output chunks of [Q^T; K^T]
MC_F = HIDDEN // 128      # 24 hidden chunks
EPS = 1e-5
NE = 578                  # q-dim padded to even (fp32r ISA requirement)
QH = [(0, 320), (320, 258)]   # q-halves (<=512 per PSUM bank, even)
VH = [(0, 512), (512, 256)]   # 768-wide output halves


def build_program():
    nc = bacc.Bacc(
        "TRN2",
        target_bir_lowering=False,
        debug=False,
        enable_asserts=False,
    )
    x_d = nc.dram_tensor("x", [128, TT, DIM], F32, kind="ExternalInput").ap()
    wqk_d = nc.dram_tensor("wqk", [128, KC, 1536], BF16, kind="ExternalInput").ap()
    wv_d = nc.dram_tensor("wv", [128, KC, DIM], BF16, kind="ExternalInput").ap()
    wproj_d = nc.dram_tensor("wproj", [128, KC, DIM], BF16, kind="ExternalInput").ap()
    wfc1_d = nc.dram_tensor("wfc1", [128, KC, HIDDEN], BF16, kind="ExternalInput").ap()
    wfc2_d = nc.dram_tensor("wfc2", [128, MC_F, DIM], BF16, kind="ExternalInput").ap()
    cqk_d = nc.dram_tensor("cqk", [128, MC_QK], F32, kind="ExternalInput").ap()
    cfc1_d = nc.dram_tensor("cfc1", [128, MC_F], F32, kind="ExternalInput").ap()
    y_d = nc.dram_tensor("y", [128, TT, DIM], F32, kind="ExternalOutput").ap()

    with tile.TileContext(nc) as tc:
        _build(tc, x_d, wqk_d, wv_d, wproj_d, wfc1_d, wfc2_d, cqk_d, cfc1_d, y_d)
    nc.compile()
    return nc


def _build(tc, x_d, wqk_d, wv_d, wproj_d, wfc1_d, wfc2_d, cqk_d, cfc1_d, y_d):
    nc = tc.nc

    def ln_tile(x_sb, t, stats, eps_sb, label):
        """LayerNorm stats+apply for one token tile -> bf16 h [128, 768]."""
        xt = x_sb[:, t, :]
        s = stats.tile([128, 1], F32, tag="s", name=f"s_{label}_{t}")
        nc.vector.reduce_sum(s[:], xt, axis=AX.X)
        sqf = stats.tile([128, DIM], F32, tag="sqf", name=f"sqf_{label}_{t}",
                         bufs=2)
        q = stats.tile([128, 1], F32, tag="q", name=f"q_{label}_{t}")
        nc.scalar.activation(sqf[:], xt, AF.Square, accum_out=q[:])
        mu = stats.tile([128, 1], F32, tag="mu", name=f"mu_{label}_{t}")
        nc.vector.tensor_scalar_mul(mu[:], s[:], 1.0 / DIM)
        var = stats.tile([128, 1], F32, tag="var", name=f"var_{label}_{t}")
        nc.vector.tensor_scalar_mul(var[:], q[:], 1.0 / DIM)
        mu2 = stats.tile([128, 1], F32, tag="mu2", name=f"mu2_{label}_{t}")
        nc.vector.tensor_mul(mu2[:], mu[:], mu[:])
        nc.vector.tensor_sub(var[:], var[:], mu2[:])
        std = stats.tile([128, 1], F32, tag="std", name=f"std_{label}_{t}")
        nc.scalar.activation(std[:], var[:], AF.Sqrt, bias=eps_sb[:])
        rstd = stats.tile([128, 1], F32, tag="rstd", name=f"rstd_{label}_{t}")
        nc.vector.reciprocal(rstd[:], std[:])
        nmr = stats.tile([128, 1], F32, tag="nmr", name=f"nmr_{label}_{t}")
        nc.vector.tensor_mul(nmr[:], mu[:], rstd[:])
        nc.vector.tensor_scalar_mul(nmr[:], nmr[:], -1.0)
        h = stats.tile([128, DIM], BF16, tag="h", name=f"h_{label}_{t}",
                       bufs=2)
        nc.scalar.activation(h[:], xt, AF.Identity, bias=nmr[:], scale=rstd[:])
        return h

    def transpose_tile(h, hT, t, tps, ident, label):
        for c in range(KC):
            ps = tps.tile([128, 128], BF16, tag="tp",
                          name=f"tp_{label}_{t}_{c}")
            nc.tensor.transpose(ps[:], h[:, 128 * c:128 * (c + 1)], ident[:])
            dst = hT[:, c, 128 * t:128 * (t + 1)]
            if c % 2 == 0:
                nc.vector.tensor_copy(dst, ps[:])
            else:
                nc.scalar.copy(dst, ps[:])

    # ---------- whole-kernel pools ----------
    with tc.tile_pool(name="const", bufs=1) as const, \
         tc.tile_pool(name="stats", bufs=2) as stats, \
         tc.tile_pool(name="pers", bufs=1) as pers:

        ident = const.tile([128, 128], BF16, tag="ident", name="ident")
        make_identity(nc, ident)
        # aux[0:64, 64:128] = I (odd-head shift), aux[64, :] = ones (denom
        # broadcast). fp32r can't be memset -> stage through F32.
        aux_f = const.tile([65, 128], F32, tag="aux_f", name="aux_f")
        nc.gpsimd.memset(aux_f[:, :], 0.0)
        make_identity(nc, aux_f[0:64, 64:128], nomemset=True)
        nc.gpsimd.memset(aux_f[64:65, :], 1.0)
        aux = const.tile([65, 128], F32R, tag="aux", name="aux")
        nc.vector.tensor_copy(aux[:], aux_f[:])
        onescol = const.tile([128, 2], F32, tag="onescol", name="onescol")
        nc.gpsimd.memset(onescol[:, 0:1], 1.0)
        nc.gpsimd.memset(onescol[:, 1:2], 0.0)
        onescol5 = const.tile([128, 2], F32, tag="onescol5", name="onescol5")
        nc.gpsimd.memset(onescol5[:, :], 0.0)
        nc.gpsimd.memset(onescol5[0:65, 0:1], 1.0)
        cqk_sb = const.tile([128, MC_QK], F32, tag="cqk", name="cqk_sb")
        nc.sync.dma_start(cqk_sb[:], cqk_d[:])
        cfc1_sb = const.tile([128, MC_F], F32, tag="cfc1", name="cfc1_sb")
        nc.sync.dma_start(cfc1_sb[:], cfc1_d[:])
        eps_sb = const.tile([128, 1], F32, tag="eps", name="eps_sb")
        nc.gpsimd.memset(eps_sb[:], EPS)

        x_sb = pers.tile([128, TT, DIM], F32, tag="x", name="x_sb")

        # wfc1/wfc2 die last among the big pools -> opened first (LIFO)
        with tc.tile_pool(name="wfc12p", bufs=1) as wfc12p:
            wfc1 = wfc12p.tile([128, KC, HIDDEN], BF16, tag="wfc1",
                               name="wfc1_sb")
            wfc2 = wfc12p.tile([128, MC_F, DIM], BF16, tag="wfc2",
                               name="wfc2_sb")

            with tc.tile_pool(name="qkvp", bufs=1) as qkvp:
                qkT_sb = qkvp.tile([128, MC_QK, BPC, NE], BF16, tag="qkT",
                                   name="qkT_sb")
                v_sb = qkvp.tile([128, BPC, NT, HEADS, HD + 2], BF16, tag="v",
                                 name="v_sb")
                for b in range(BPC):
                    for t in range(NT):
                        src_col = onescol if t < NT - 1 else onescol5
                        nc.vector.tensor_copy(
                            v_sb[:, b, t, :, HD:HD + 2],
                            src_col[:, None, :].to_broadcast([128, HEADS, 2]))

                # ------------- LN1 + h^T + QKV -------------
                with tc.tile_pool(name="hTp", bufs=1) as hTp, \
                     tc.tile_pool(name="wqkvp", bufs=1) as wqkvp, \
                     tc.tile_pool(name="ln1ps", bufs=2, space="PSUM") as tps1, \
                     tc.tile_pool(name="qkvps", bufs=6, space="PSUM") as qps:
                    hT = hTp.tile([128, KC, TOKP], BF16, tag="hT", name="hT1")
                    wqk = wqkvp.tile([128, KC, 1536], BF16, tag="wqk",
                                     name="wqk_sb")
                    wv = wqkvp.tile([128, KC, DIM], BF16, tag="wv",
                                    name="wv_sb")
                    # x per-tile DMAs so LN1 pipelines with the load
                    for t in range(TT):
                        nc.sync.dma_start(x_sb[:, t, :], x_d[:, t, :])
                    nc.sync.dma_start(wqk[:], wqk_d[:])
                    nc.sync.dma_start(wv[:], wv_d[:])
                    for t in range(TT):
                        h = ln_tile(x_sb, t, stats, eps_sb, "ln1")
                        transpose_tile(h, hT, t, tps1, ident, "ln1")

                    # weights for later phases: DMAs run during QKV/attention
                    nc.sync.dma_start(wfc1[:], wfc1_d[:])
                    nc.sync.dma_start(wfc2[:], wfc2_d[:])

                    for m in range(MC_QK):
                        for b in range(BPC):
                            for (q0, qw) in QH:
                                ps = qps.tile([128, 512], F32, tag="qk",
                                              name=f"qkps_{m}_{b}_{q0}")
                                for c in range(KC):
                                    nc.tensor.matmul(
                                        ps[:, :qw],
                                        wqk[:, c, 128 * m:128 * (m + 1)],
                                        hT[:, c, NB * b + q0:NB * b + q0 + qw],
                                        start=(c == 0), stop=(c == KC - 1),
                                    )
                                nc.scalar.activation(
                                    qkT_sb[:, m, b, q0:q0 + qw], ps[:, :qw],
                                    AF.Identity, bias=cqk_sb[:, m:m + 1],
                                )
                    for b in range(BPC):
                        for t in range(NT):
                            for (o0, ow) in VH:
                                ps = qps.tile([128, 512], F32, tag="qk",
                                              name=f"vps_{b}_{t}_{o0}")
                                for c in range(KC):
                                    nc.tensor.matmul(
                                        ps[:, :ow],
                                        hT[:, c, NB * b + 128 * t:
                                           NB * b + 128 * (t + 1)],
                                        wv[:, c, o0:o0 + ow],
                                        start=(c == 0), stop=(c == KC - 1),
                                    )
                                ps3 = ps[:, :ow].rearrange(
                                    "p (h d) -> p h d", d=HD)
                                nc.vector.tensor_copy(
                                    v_sb[:, b, t, o0 // HD:(o0 + ow) // HD,
                                         0:HD], ps3)

                # ------------- attention -------------
                with tc.tile_pool(name="oTp", bufs=1) as oTp, \
                     tc.tile_pool(name="wprojp", bufs=1) as wprojp:
                    oT_sb = oTp.tile([128, KC, BPC * NE], BF16, tag="oT",
                                     name="oT_sb")
                    wproj = wprojp.tile([128, KC, DIM], BF16, tag="wproj",
                                        name="wproj_sb")
                    nc.sync.dma_start(wproj[:], wproj_d[:])

                    with tc.tile_pool(name="esp", bufs=2) as esp, \
                         tc.tile_pool(name="attsmall", bufs=2) as asml, \
                         tc.tile_pool(name="attps", bufs=2, space="PSUM") as aps:

                        def attn_head(b, h):
                            pbase = 64 * (h % 2)
                            cQ = h // 2
                            cK = 6 + h // 2
                            es = esp.tile([128, NT, NE], BF16, tag="es",
                                          name=f"es_{b}_{h}")
                            pva = aps.tile([66, 320], F32, tag="pva",
                                           name=f"pva_{b}_{h}")
                            pvb = aps.tile([66, 258], F32, tag="pvb",
                                           name=f"pvb_{b}_{h}")
                            for kt in range(NT):
                                kw = 128 if kt < 4 else 66
                                for (q0, qw), stag in zip(QH, ("sa", "sb")):
                                    sps = aps.tile([128, qw], F32, tag=stag,
                                                   name=f"sps_{b}_{h}_{kt}_{q0}")
                                    nc.tensor.matmul(
                                        sps[:kw, :],
                                        qkT_sb[pbase:pbase + 64, cK, b,
                                               128 * kt:128 * kt + kw],
                                        qkT_sb[pbase:pbase + 64, cQ, b,
                                               q0:q0 + qw],
                                        start=True, stop=True,
                                    )
                                    nc.scalar.activation(
                                        es[:kw, kt, q0:q0 + qw], sps[:kw, :],
                                        AF.Exp)
                                for (q0, qw) in QH:
                                    pv = pva if q0 == 0 else pvb
                                    nc.tensor.matmul(
                                        pv[:, :],
                                        v_sb[0:kw, b, kt, h, :],
                                        es[0:kw, kt, q0:q0 + qw],
                                        start=(kt == 0), stop=(kt == NT - 1),
                                    )

                            def tail():
                                col0 = NE * b
                                # "dot": row 64 = denominator; odd heads also
                                # stage rows 0:64 here for the shift-matmul
                                dot = asml.tile([66, NE], F32R, tag="dot",
                                                name=f"dot_{b}_{h}")
                                if h % 2 == 0:
                                    nc.vector.tensor_copy(dot[64:65, 0:320],
                                                          pva[64:65, :])
                                    nc.vector.tensor_copy(dot[64:65, 320:NE],
                                                          pvb[64:65, :])
                                else:
                                    nc.vector.tensor_copy(dot[:, 0:320],
                                                          pva[:, :])
                                    nc.vector.tensor_copy(dot[:, 320:NE],
                                                          pvb[:, :])
                                rr = asml.tile([128, NE], F32, tag="rr",
                                               name=f"rr_{b}_{h}")
                                for (q0, qw), stag in zip(QH, ("sa", "sb")):
                                    rps = aps.tile([128, qw], F32, tag=stag,
                                                   name=f"rps_{b}_{h}_{q0}")
                                    nc.tensor.matmul(
                                        rps[:], aux[64:65, :],
                                        dot[64:65, q0:q0 + qw],
                                        start=True, stop=True,
                                    )
                                    nc.vector.reciprocal(
                                        rr[:, q0:q0 + qw], rps[:])
                                if h % 2 == 0:
                                    for (q0, qw) in QH:
                                        pv = pva if q0 == 0 else pvb
                                        nc.vector.tensor_mul(
                                            oT_sb[0:64, cQ,
                                                  col0 + q0:col0 + q0 + qw],
                                            pv[0:64, :], rr[0:64, q0:q0 + qw])
                                else:
                                    for (q0, qw), stag in zip(QH, ("sa", "sb")):
                                        shps = aps.tile(
                                            [128, qw], F32, tag=stag,
                                            name=f"shps_{b}_{h}_{q0}")
                                        nc.tensor.matmul(
                                            shps[:], aux[0:64, :],
                                            dot[0:64, q0:q0 + qw],
                                            start=True, stop=True,
                                        )
                                        nc.vector.tensor_mul(
                                            oT_sb[64:128, cQ,
                                                  col0 + q0:col0 + q0 + qw],
                                            shps[64:128, :],
                                            rr[64:128, q0:q0 + qw])
                            return tail

                        prev_tail = None
                        for b in range(BPC):
                            for h in range(HEADS):
                                t = attn_head(b, h)
                                if prev_tail is not None:
                                    prev_tail()
                                prev_tail = t
                        prev_tail()

                    # ------------- proj + residual (in-place into x) ------
                    with tc.tile_pool(name="projps", bufs=3,
                                      space="PSUM") as pps:
                        for b in range(BPC):
                            for t in range(NT):
                                tw = 128 if t < 4 else 66
                                col0 = NE * b + 128 * t
                                for (o0, ow) in VH:
                                    ps = pps.tile([128, 512], F32, tag="pj",
                                                  name=f"pjps_{b}_{t}_{o0}")
                                    for c in range(KC):
                                        nc.tensor.matmul(
                                            ps[:tw, :ow],
                                            oT_sb[:, c, col0:col0 + tw],
                                            wproj[:, c, o0:o0 + ow],
                                            start=(c == 0), stop=(c == KC - 1),
                                        )
                                    xs = x_sb[:tw, NT * b + t, o0:o0 + ow]
                                    nc.vector.tensor_add(xs, ps[:tw, :ow], xs)

                # ------------- LN2 + h^T -------------
                with tc.tile_pool(name="hT2p", bufs=1) as hT2p:
                    hT2 = hT2p.tile([128, KC, TOKP], BF16, tag="hT",
                                    name="hT2")
                    with tc.tile_pool(name="ln2ps", bufs=4,
                                      space="PSUM") as tps2:
                        for t in range(TT):
                            h = ln_tile(x_sb, t, stats, eps_sb, "ln2")
                            transpose_tile(h, hT2, t, tps2, ident, "ln2")

                    # ------------- MLP -------------
                    with tc.tile_pool(name="fp", bufs=3) as fp, \
                         tc.tile_pool(name="outp", bufs=2) as outp, \
                         tc.tile_pool(name="f1ps", bufs=3, space="PSUM") as f1ps, \
                         tc.tile_pool(name="f2ps", bufs=1, space="PSUM") as f2ps:
                        for p in range(NT):  # 5 token-tile pairs
                            fc2ps = [
                                [f2ps.tile([128, w], F32, tag=f"f2_{j}_{o0}",
                                           name=f"f2ps_{p}_{j}_{o0}")
                                 for (o0, w) in VH]
                                for j in range(2)
                            ]
                            for m in range(MC_F):
                                ps1 = f1ps.tile([128, 256], F32, tag="f1",
                                                name=f"f1ps_{p}_{m}")
                                for c in range(KC):
                                    nc.tensor.matmul(
                                        ps1[:],
                                        wfc1[:, c, 128 * m:128 * (m + 1)],
                                        hT2[:, c, 256 * p:256 * (p + 1)],
                                        start=(c == 0), stop=(c == KC - 1),
                                    )
                                fpr = fp.tile([128, 256], BF16, tag="fpr",
                                              name=f"fpr_{p}_{m}")
                                nc.scalar.activation(fpr[:], ps1[:], AF.Gelu,
                                                     bias=cfc1_sb[:, m:m + 1])
                                for j in range(2):
                                    for hi, (o0, ow) in enumerate(VH):
                                        nc.tensor.matmul(
                                            fc2ps[j][hi][:],
                                            fpr[:, 128 * j:128 * (j + 1)],
                                            wfc2[:, m, o0:o0 + ow],
                                            start=(m == 0),
                                            stop=(m == MC_F - 1),
                                        )
                            for j in range(2):
                                t_idx = 2 * p + j
                                for hi, (o0, ow) in enumerate(VH):
                                    ot = outp.tile([128, 512], F32, tag="out",
                                                   name=f"out_{p}_{j}_{o0}")
                                    nc.vector.tensor_add(
                                        ot[:, :ow], fc2ps[j][hi][:],
                                        x_sb[:, t_idx, o0:o0 + ow])
                                    nc.sync.dma_start(
                                        y_d[:, t_idx, o0:o0 + ow], ot[:, :ow])


_PROGRAM_CACHE = {}


def _get_program():
    if "nc" not in _PROGRAM_CACHE:
        _PROGRAM_CACHE["nc"] = build_program()
    return _PROGRAM_CACHE["nc"]


def prep_inputs(x, ln1_g, ln1_b, w_qkv, b_qkv, w_proj, b_proj,
                ln2_g, ln2_b, w_fc1, b_fc1, w_fc2, b_fc2):
    """Host-side exact preprocessing -> per-core input maps."""
    f = np.float32
    bf = ml_dtypes.bfloat16
    ln1_g = np.asarray(ln1_g, f); ln1_b = np.asarray(ln1_b, f)
    ln2_g = np.asarray(ln2_g, f); ln2_b = np.asarray(ln2_b, f)
    w_qkv = np.asarray(w_qkv, f); b_qkv = np.asarray(b_qkv, f)
    w_proj = np.asarray(w_proj, f); b_proj = np.asarray(b_proj, f)
    w_fc1 = np.asarray(w_fc1, f); b_fc1 = np.asarray(b_fc1, f)
    w_fc2 = np.asarray(w_fc2, f); b_fc2 = np.asarray(b_fc2, f)

    wqkv_g = ln1_g[:, None] * w_qkv
    wqkv_g[:, :DIM] *= f(0.125)  # attention scale 1/sqrt(64), exact
    cqkv = ln1_b @ w_qkv + b_qkv
    cqkv[:DIM] *= f(0.125)
    wqk = np.ascontiguousarray(
        wqkv_g[:, :1536].reshape(KC, 128, 1536).transpose(1, 0, 2)).astype(bf)
    wv = np.ascontiguousarray(
        wqkv_g[:, 1536:].reshape(KC, 128, DIM).transpose(1, 0, 2)).astype(bf)
    cqk = np.ascontiguousarray(cqkv[:1536].reshape(MC_QK, 128).T)
    if not np.allclose(cqkv[1536:], 0.0):
        raise NotImplementedError("nonzero V bias not supported on device path")
    if not np.allclose(b_proj, 0.0) or not np.allclose(b_fc2, 0.0):
        raise NotImplementedError("nonzero proj/fc2 bias not supported")

    wproj = np.ascontiguousarray(
        w_proj.reshape(KC, 128, DIM).transpose(1, 0, 2)).astype(bf)

    wfc1_g = ln2_g[:, None] * w_fc1
    cfc1 = (ln2_b @ w_fc1 + b_fc1).astype(f)
    wfc1 = np.ascontiguousarray(
        wfc1_g.reshape(KC, 128, HIDDEN).transpose(1, 0, 2)).astype(bf)
    cfc1_l = np.ascontiguousarray(cfc1.reshape(MC_F, 128).T)
    wfc2 = np.ascontiguousarray(
        w_fc2.reshape(MC_F, 128, DIM).transpose(1, 0, 2)).astype(bf)

    x = np.asarray(x, f)
    in_maps = []
    for core in range(CORES):
        xs = x[core * BPC:(core + 1) * BPC]  # [2, 577, 768]
        xp = np.zeros((BPC, NB, DIM), f)
        xp[:, :N, :] = xs
        xl = np.ascontiguousarray(
            xp.reshape(TT, 128, DIM).transpose(1, 0, 2))  # [128, 10, 768]
        in_maps.append({
            "x": xl, "wqk": wqk, "wv": wv, "wproj": wproj,
            "wfc1": wfc1, "wfc2": wfc2, "cqk": cqk, "cfc1": cfc1_l,
        })
    return in_maps


def assemble_output(results):
    """results: list of 8 dicts with 'y' [128, 10, 768] -> [16, 577, 768]."""
    outs = []
    for core in range(CORES):
        yl = np.asarray(results[core]["y"])
        yp = yl.transpose(1, 0, 2).reshape(BPC, NB, DIM)
        outs.append(yp[:, :N, :])
    return np.concatenate(outs, axis=0).astype(np.float32)


def kernel(**inputs):
    from concourse.bass_utils import run_bass_kernel_spmd

    nc = _get_program()
    in_maps = prep_inputs(**inputs)
    res = run_bass_kernel_spmd(nc, in_maps, list(range(CORES)))
    return assemble_output(res.results)


if __name__ == "__main__":
    nc = build_program()
    print("compiled ok")
